# revision 11
# baseline (speedup 1.0000x reference)
"""Multi-head attention (B=2, S=2048, D=1024, H=16) on 8 trn2 NeuronCores.

Sharding: core c handles batch b = c//4 and heads 4*(c%4) .. 4*(c%4)+4
(tensor-parallel over heads, data-parallel over batch). Each core computes
its 4 heads' contribution to the output projection; the host sums the 4
partials per batch element and adds wo_b.

Layout strategy (all "T" tensors have the contraction dim on partitions):
  - host pre-transposes q,k,v -> qT/kT/vT (D, S) and mask -> binary bf16
    maskT (Sk, Sq), so the device never transposes anything.
  - projections produce qT_pair/kT_pair (128 = 2 heads x 64, Sq) and
    vp (Sk, 4 heads x [64 v-cols + ones-col]).
  - scores are computed transposed: alphaT (Sk-chunk, Sq-chunk) via two
    row-tiled K=64 matmuls (tile_position packs 2 heads onto the PE array).
  - softmax without max-subtraction (values are small): exp on ScalarE
    (PSUM -> SBUF bf16), binary-mask multiply on VectorE, and the softmax
    denominator comes free as row 64 of the PV matmul (ones column in vp).
  - PV: xT(65, Sq-chunk) accumulated over 16 Sk chunks in PSUM.
  - normalize with reciprocal + gpsimd partition_broadcast, then the output
    projection contracts 2 heads per matmul, accumulating both pairs.
"""

import numpy as np

B, S, D, H = 2, 2048, 1024, 16
DH = D // H  # 64
HEADS_PER_CORE = 4
N_CORES = 8
NQ = 4  # Sq chunks of 512
NSK = 16  # Sk chunks of 128
KC = 8  # D chunks of 128

_NC = None  # cached compiled bass program


def _build():
    import concourse.mybir as mybir
    import concourse.tile as tile
    from concourse import bacc

    F32 = mybir.dt.float32
    BF16 = mybir.dt.bfloat16
    P = 128

    nc = bacc.Bacc("TRN2")

    qT = nc.dram_tensor("qT", [D, S], F32, kind="ExternalInput")
    kT = nc.dram_tensor("kT", [D, S], F32, kind="ExternalInput")
    vT = nc.dram_tensor("vT", [D, S], F32, kind="ExternalInput")
    maskT = nc.dram_tensor("maskT", [S, S], BF16, kind="ExternalInput")
    wqT = nc.dram_tensor("wqT", [D, 256], F32, kind="ExternalInput")
    wkT = nc.dram_tensor("wkT", [D, 256], F32, kind="ExternalInput")
    wvT = nc.dram_tensor("wvT", [D, 256], F32, kind="ExternalInput")
    woT = nc.dram_tensor("woT", [256, D], F32, kind="ExternalInput")
    wqb = nc.dram_tensor("wqb", [256], F32, kind="ExternalInput")
    wkb = nc.dram_tensor("wkb", [256], F32, kind="ExternalInput")
    wvb = nc.dram_tensor("wvb", [256], F32, kind="ExternalInput")
    out = nc.dram_tensor("out", [S, D], F32, kind="ExternalOutput")

    AF = mybir.ActivationFunctionType
    MUL = mybir.AluOpType.mult
    ADD = mybir.AluOpType.add

    with tile.TileContext(nc) as tc:
        with (
            tc.tile_pool(name="persist", bufs=1) as persist,
            tc.tile_pool(name="vstream", bufs=10) as vstream,
            tc.tile_pool(name="qstream", bufs=3) as qstream,
            tc.tile_pool(name="pbuf", bufs=3) as pbuf,
            tc.tile_pool(name="obuf", bufs=3) as obuf,
            tc.tile_pool(name="nbuf", bufs=4) as nbuf,
        ):
            # ---- weights + biases ----
            wvT_sb = persist.tile([P, KC, 256], F32, tag="wvT")
            for kc in range(KC):
                nc.sync.dma_start(
                    wvT_sb[:, kc], wvT[P * kc : P * (kc + 1), :]
                )
            wvb_sb = persist.tile([P, 256], F32, tag="wvb")
            nc.sync.dma_start(wvb_sb[:], wvb[:][None, :].to_broadcast((P, 256)))

            with (
                tc.tile_pool(name="ps_proj", bufs=2, space="PSUM") as ps_proj,
                tc.tile_pool(name="ps_alpha", bufs=2, space="PSUM") as ps_alpha,
                tc.tile_pool(name="ps_xp", bufs=2, space="PSUM") as ps_xp,
            ):
                # ---- v projection: vp[sk] (128 Sk, 4 heads x 64) bf16 ----
                vp_sb = []
                for sk in range(NSK):
                    vp = persist.tile([P, 4, 65], BF16, tag=f"vp{sk}", name=f"vp{sk}")
                    nc.gpsimd.memset(vp[:], 1.0)  # ones column (col 64 per head)
                    vp_sb.append(vp)
                for sk2 in range(NSK // 2):
                    vsls = [
                        vstream.tile([P, 256], F32, tag="vsl", name=f"vsl{sk2}_{kc}")
                        for kc in range(KC)
                    ]
                    for kc in range(KC):
                        nc.sync.dma_start(
                            vsls[kc][:],
                            vT[P * kc : P * (kc + 1), 256 * sk2 : 256 * (sk2 + 1)],
                        )
                    for skl in range(2):
                        sk = 2 * sk2 + skl
                        vp_ps = ps_proj.tile(
                            [P, 512], F32, tag="psproj", name=f"vpps{sk}"
                        )
                        for kc in range(KC):
                            nc.tensor.matmul(
                                vp_ps[:, 0:256],
                                vsls[kc][:, P * skl : P * (skl + 1)],
                                wvT_sb[:, kc],
                                start=(kc == 0),
                                stop=(kc == KC - 1),
                            )
                        # add bias and cast to bf16
                        nc.vector.tensor_tensor(
                            vp_sb[sk][:, :, 0:64],
                            vp_ps[:, 0:256].rearrange("p (h d) -> p h d", h=4),
                            wvb_sb[:].rearrange("p (h d) -> p h d", h=4),
                            ADD,
                        )

                # ---- remaining weights (needed after vproj) ----
                wqT_sb = persist.tile([P, KC, 256], F32, tag="wqT")
                nc.sync.dma_start(
                    wqT_sb[:], wqT[:].rearrange("(kc p) m -> p kc m", p=P)
                )
                wkT_sb = persist.tile([P, KC, 256], F32, tag="wkT")
                nc.sync.dma_start(
                    wkT_sb[:], wkT[:].rearrange("(kc p) m -> p kc m", p=P)
                )
                woT_sb = persist.tile([P, 2, D], F32, tag="woT")
                nc.sync.dma_start(
                    woT_sb[:], woT[:].rearrange("(pr p) m -> p pr m", p=P)
                )
                wqb_sb = persist.tile([P, 2], F32, tag="wqb")
                nc.sync.dma_start(wqb_sb[:], wqb[:].rearrange("(pr p) -> p pr", p=P))
                wkb_sb = persist.tile([P, 2], F32, tag="wkb")
                nc.sync.dma_start(wkb_sb[:], wkb[:].rearrange("(pr p) -> p pr", p=P))

                # ---- q/k projections -> per-chunk qT/kT pair tiles ----
                qTp = [
                    [
                        persist.tile(
                            [P, 512], F32, tag=f"qTp{p}_{nq}", name=f"qTp{p}_{nq}"
                        )
                        for nq in range(NQ)
                    ]
                    for p in range(2)
                ]
                kTp = [
                    [
                        persist.tile(
                            [P, 512], F32, tag=f"kTp{p}_{nq}", name=f"kTp{p}_{nq}"
                        )
                        for nq in range(NQ)
                    ]
                    for p in range(2)
                ]
                for src, wsb, bsb, dst, nm in (
                    (qT, wqT_sb, wqb_sb, qTp, "q"),
                    (kT, wkT_sb, wkb_sb, kTp, "k"),
                ):
                    for nq in range(NQ):
                        pps = [
                            ps_proj.tile(
                                [P, 512], F32, tag="psproj", name=f"{nm}ps{nq}_{p}"
                            )
                            for p in range(2)
                        ]
                        for kc in range(KC):
                            xsl = qstream.tile(
                                [P, 512], F32, tag="xsl", name=f"{nm}sl{nq}_{kc}"
                            )
                            nc.sync.dma_start(
                                xsl[:],
                                src[P * kc : P * (kc + 1), 512 * nq : 512 * (nq + 1)],
                            )
                            for p in range(2):
                                nc.tensor.matmul(
                                    pps[p][:],
                                    wsb[:, kc, 128 * p : 128 * (p + 1)],
                                    xsl[:],
                                    start=(kc == 0),
                                    stop=(kc == KC - 1),
                                )
                        for p in range(2):
                            # psum -> sbuf with per-partition bias add
                            nc.scalar.activation(
                                dst[p][nq][:],
                                pps[p][:],
                                AF.Identity,
                                bias=bsb[:, p : p + 1],
                            )

                # ---- mask tiles (binary bf16, [Sk-part, Sq]) ----
                mask_sb = []
                for sk in range(NSK):
                    m = persist.tile([P, S], BF16, tag=f"mask{sk}", name=f"mask{sk}")
                    nc.sync.dma_start(m[:], maskT[P * sk : P * (sk + 1), :])
                    mask_sb.append(m)

                # ---- attention + normalization + output projection ----
                xnorm = [
                    [
                        persist.tile(
                            [P, 512], F32, tag=f"xn{p}_{nq}", name=f"xn{p}_{nq}"
                        )
                        for nq in range(NQ)
                    ]
                    for p in range(2)
                ]
                for nq in range(NQ):
                    for pr in range(2):
                        xps = [
                            ps_xp.tile(
                                [P, 512], F32, tag="xps", name=f"xps{nq}_{pr}_{h}"
                            )
                            for h in range(2)
                        ]
                        for sk in range(NSK):
                            alpha = ps_alpha.tile(
                                [P, 1024], F32, tag="alpha", name=f"al{nq}_{pr}_{sk}"
                            )
                            for h in range(2):
                                nc.tensor.matmul(
                                    alpha[:, 512 * h : 512 * (h + 1)],
                                    kTp[pr][sk // 4][
                                        64 * h : 64 * h + 64,
                                        P * (sk % 4) : P * (sk % 4 + 1),
                                    ],
                                    qTp[pr][nq][64 * h : 64 * h + 64, :],
                                    start=True,
                                    stop=True,
                                    tile_position=(64 * h, 0),
                                )
                            psb = pbuf.tile(
                                [P, 1024],
                                BF16,
                                tag="psb",
                                name=f"psb{nq}_{pr}_{sk}",
                            )
                            nc.scalar.activation(psb[:], alpha[:], AF.Exp)
                            nc.vector.tensor_tensor(
                                psb[:].rearrange("p (h n) -> p h n", h=2),
                                psb[:].rearrange("p (h n) -> p h n", h=2),
                                mask_sb[sk][:, 512 * nq : 512 * (nq + 1)][
                                    :, None, :
                                ].to_broadcast((P, 2, 512)),
                                MUL,
                            )
                            for h in range(2):
                                nc.tensor.matmul(
                                    xps[h][0:65, :],
                                    vp_sb[sk][:, 2 * pr + h],
                                    psb[:, 512 * h : 512 * (h + 1)],
                                    start=(sk == 0),
                                    stop=(sk == NSK - 1),
                                )
                        for h in range(2):
                            r = nbuf.tile(
                                [1, 512], F32, tag="r", name=f"r{nq}_{pr}_{h}"
                            )
                            nc.vector.reciprocal(r[:], xps[h][64:65, :])
                            rb = nbuf.tile(
                                [64, 512], F32, tag="rb", name=f"rb{nq}_{pr}_{h}"
                            )
                            nc.gpsimd.partition_broadcast(rb[:], r[:])
                            nc.vector.tensor_tensor(
                                xnorm[pr][nq][64 * h : 64 * h + 64, :],
                                xps[h][0:64, :],
                                rb[:],
                                MUL,
                            )

                # ---- output projection ----
                for m in range(S // P):
                    osb = obuf.tile([P, D], F32, tag="osb", name=f"osb{m}")
                    for d in range(2):
                        ops = ps_proj.tile(
                            [P, 512], F32, tag="psproj", name=f"ops{m}_{d}"
                        )
                        for pr in range(2):
                            nc.tensor.matmul(
                                ops[:],
                                xnorm[pr][m // 4][:, P * (m % 4) : P * (m % 4 + 1)],
                                woT_sb[:, pr, 512 * d : 512 * (d + 1)],
                                start=(pr == 0),
                                stop=(pr == 1),
                            )
                        nc.any.tensor_copy(out=osb[:, 512 * d : 512 * (d + 1)], in_=ops[:])
                    nc.sync.dma_start(out[P * m : P * (m + 1), :], osb[:])

    nc.finalize()
    return nc


def _get_nc():
    global _NC
    if _NC is None:
        _NC = _build()
    return _NC


def _prep_inputs(q, k, v, mask, wq_w, wq_b, wk_w, wk_b, wv_w, wv_b, wo_w, wo_b):
    import ml_dtypes

    f32 = np.float32
    q = np.asarray(q, f32)
    k = np.asarray(k, f32)
    v = np.asarray(v, f32)
    mask = np.asarray(mask)
    wq_w = np.asarray(wq_w, f32)
    wk_w = np.asarray(wk_w, f32)
    wv_w = np.asarray(wv_w, f32)
    wo_w = np.asarray(wo_w, f32)

    qTb = [np.ascontiguousarray(q[b].T) for b in range(B)]
    kTb = [np.ascontiguousarray(k[b].T) for b in range(B)]
    vTb = [np.ascontiguousarray(v[b].T) for b in range(B)]
    maskTb = [
        np.ascontiguousarray((~mask[b, 0]).T).astype(ml_dtypes.bfloat16)
        for b in range(B)
    ]

    in_maps = []
    for c in range(N_CORES):
        b = c // 4
        g = c % 4
        rows = slice(256 * g, 256 * (g + 1))
        in_maps.append(
            {
                "qT": qTb[b],
                "kT": kTb[b],
                "vT": vTb[b],
                "maskT": maskTb[b],
                "wqT": np.ascontiguousarray(wq_w[rows, :].T),
                "wkT": np.ascontiguousarray(wk_w[rows, :].T),
                "wvT": np.ascontiguousarray(wv_w[rows, :].T),
                "woT": np.ascontiguousarray(wo_w[:, rows].T),
                "wqb": np.ascontiguousarray(np.asarray(wq_b, f32)[rows]),
                "wkb": np.ascontiguousarray(np.asarray(wk_b, f32)[rows]),
                "wvb": np.ascontiguousarray(np.asarray(wv_b, f32)[rows]),
            }
        )
    return in_maps


def run(inputs, trace=False):
    """Run the kernel; returns (output, BassKernelResults)."""
    from concourse.bass_utils import run_bass_kernel_spmd

    in_maps = _prep_inputs(**inputs)
    nc = _get_nc()
    res = None
    last_exc = None
    for attempt in range(3):
        try:
            res = run_bass_kernel_spmd(
                nc, in_maps, core_ids=list(range(N_CORES)), trace=trace
            )
            break
        except Exception as e:  # transient device/tunnel failures
            last_exc = e
            try:
                import jax

                jax.clear_caches()
                try:
                    jax.extend.backend.clear_backends()
                except Exception:
                    from jax._src import api as _jax_api

                    _jax_api.clear_backends()
            except Exception:
                pass
            import time as _time

            _time.sleep(2.0 * (attempt + 1))
    if res is None:
        raise last_exc
    wo_b = np.asarray(inputs["wo_b"], np.float32)
    out = np.zeros((B, S, D), np.float32)
    for b in range(B):
        acc = np.zeros((S, D), np.float32)
        for g in range(4):
            acc += res.results[4 * b + g]["out"]
        out[b] = acc + wo_b[None, :]
    return out, res


def kernel(**inputs) -> np.ndarray:
    out, _ = run(inputs, trace=False)
    return out
